# revision 16
# baseline (speedup 1.0000x reference)
"""Trainium2 Bass kernel for multi-head self-attention (dense transformer block).

Problem: x[4, 2048, 1024], w_qkv[3072, 1024], w_out[1024, 1024], b_out[1024]
  qkv = x @ w_qkv.T, rearranged 'b t (d k h) -> k b h t d' (k=3, h=16)
  attn = softmax(q @ k.T * DIM**-0.5); out = (attn @ v) concat heads @ w_out.T + b_out

Sharding (8 cores): data-parallel over batch b (4) x tensor-parallel over
head-groups (2 groups of 8 heads).  Each core gets x[b] (bf16, transposed on
host), its head-group's qkv weight columns (bf16; q/k columns PERMUTED so the
projection directly emits the fp8 DoubleRow operand layout), and the matching
w_out columns; it produces a partial [T, DIM] fp32 output which the host sums
per batch pair and adds b_out.

Device-side dataflow per core (8 heads, T=2048, DH=64):
  - K/Q projections (bf16) write PSUM column-blocks (g2, kt) whose partition
    is h4*32+dlo; DVE fp32->fp8e4 copies land them in q8/k8
    [32*h4+dlo, g2, kt, t] - the [32, 2, *] layout MatmulPerfMode.DoubleRow
    wants, so scores run at 0.5 cycles/row (2x the bf16 contraction-64 path).
  - scores S^T[j, i] per head: 16 j-blocks in groups [6, 6, 4] per 3-bank
    PSUM tile; ScalarE exp (softmax scale folded in) -> bf16 e tiles.
  - AV: e.T @ [v | 1] per (head, 128-query block) in bf16; the ones column
    gives the softmax denominator; DVE reciprocal+scale -> attn bf16.
  - attn -> attnT via DMA-transpose (XBAR); bf16 out-proj against woT;
    fp32 partial DMA'd out.

Schedule: phase A streams K(g2=0)+Q0 (~17us) so ScalarE's exp stream starts
early; K(g2=1) rides units 0-3 and V units 4-11 (re-streamed x); AVs lag
behind a 13-deep e-tile ring (which recycles the retired wk/wv buffers);
transposes/projection trail one chunk behind.
"""

import math
from contextlib import ExitStack
from dataclasses import dataclass

import numpy as np

import concourse.bass as bass
import concourse.mybir as mybir
import concourse.tile as tile
from concourse import bacc
from concourse.bass_utils import run_bass_kernel_spmd

F32 = mybir.dt.float32
BF16 = mybir.dt.bfloat16
F16 = mybir.dt.float16
FP8 = mybir.dt.float8e4
P = 128
DR = mybir.MatmulPerfMode.DoubleRow

# ---- custom DVE exp: e^(s*SCALE) = (min((p(s))^4, cap))^16 with p a cubic
# fit of 2^(s*SCALE*log2e/64).  Two custom DVE ops (cubic+2 squares, then
# clamp+4 squares) offload ~25% of the softmax exp from ScalarE to DVE.
# Coefficients fit in the raw-score domain; c0 == 1.0 rides Src1 (a ones
# tile) because the TTSS struct has only three scalar slots.
EXP_C3 = 1.8656129044922743e-11
EXP_C2 = 1.1971716705341052e-07
EXP_C1 = 0.000488392551438451
EXP_CAP = 2.0 ** (15.5 / 16.0)


def _register_dve_exp_ops():
    from concourse import dve_ops
    from concourse.dve_spec import Spec, Src0, Src1, C0, C1, C2, sq, minn, _has_src1, lower
    from concourse.dve_uop import DveOpSpec

    if "EXP2_POLY_ANT" in dve_ops._SUB_OPCODE_FOR_NAME:
        by_name = {op.name: op for op in dve_ops.OPS}
        return by_name["EXP2_POLY_ANT"], by_name["EXP2_FIN_ANT"]

    def make_op(name, spec_body, ref, row):
        spec = Spec(body=spec_body, reference=ref)
        shas = {
            ver: DveOpSpec(
                name=name, opcode=row, uops=lower(spec, ver=ver),
                rd1_en=_has_src1(spec),
            ).sha(ver)
            for ver in ("v3", "v4")
        }
        op = dve_ops.DveOp(name, spec, False, shas)
        dve_ops.OPS.append(op)
        dve_ops._SUB_OPCODE_FOR_NAME[name] = row
        return op

    op1 = make_op(
        "EXP2_POLY_ANT",
        sq(sq(((Src0 * C0 + C1) * Src0 + C2) * Src0 + Src1)),
        lambda in0, in1, s0, s1, imm2: (
            (((in0.astype(np.float32) * s0 + s1) * in0 + imm2) * in0 + in1) ** 4
        ),
        17,
    )
    op2 = make_op(
        "EXP2_FIN_ANT",
        sq(sq(sq(sq(minn(Src0, C0))))),
        lambda in0, in1, s0, s1, imm2: np.minimum(in0.astype(np.float32), s0) ** 16,
        18,
    )
    return op1, op2


EXP2_POLY_ANT, EXP2_FIN_ANT = _register_dve_exp_ops()


@dataclass(frozen=True)
class Cfg:
    T: int = 2048      # sequence length
    DIM: int = 1024    # model dim (= qkv contraction dim)
    NH: int = 8        # heads per core
    DH: int = 64       # head dim
    SCALE: float = 1024.0 ** -0.5

    @property
    def CB(self):      # contraction blocks of 128 over DIM
        return self.DIM // P

    @property
    def OD(self):      # per-core attention width = NH*DH
        return self.NH * self.DH

    @property
    def OB(self):      # o-blocks of 128 over OD
        return self.OD // P

    @property
    def JB(self):      # key blocks of 128
        return self.T // P

    @property
    def ICSZ(self):    # query chunk size
        return 256

    @property
    def NIC(self):     # number of query chunks
        return self.T // self.ICSZ

    @property
    def IB(self):      # query blocks of 128 per chunk
        return self.ICSZ // P

    @property
    def TCH(self):     # t-chunk for projection rhs streaming
        return 256

    @property
    def NTCH(self):
        return self.T // self.TCH


def _emit_kernel(tc, cfg, xT, wq, wk, wv, woT, out):
    nc = tc.nc
    c = cfg
    VW = c.DH + 1          # per-head V width incl. ones column
    JGROUPS = [6, 6, 4]    # j-blocks per scores PSUM tile / exp instruction

    ctx = ExitStack()
    with ctx:
        persist = ctx.enter_context(tc.tile_pool(name="persist", bufs=1))
        mmp = ctx.enter_context(tc.tile_pool(name="mmp", bufs=2, space="PSUM"))
        smp = ctx.enter_context(tc.tile_pool(name="smp", bufs=2, space="PSUM"))

        # fp8 q/k in DoubleRow layout: [32*h4+dlo, g2, kt, t]; h=g2*4+h4,
        # d=kt*32+dlo
        q8 = persist.tile([P, 2, 2, c.T], FP8, name="q8", tag="q8")
        k8 = persist.tile([P, 2, 2, c.T], FP8, name="k8", tag="k8")
        v_sb = persist.tile([P, c.JB, c.NH, VW], BF16, name="v_sb", tag="v")
        woT_sb = persist.tile([P, c.OB, c.DIM], BF16, name="woT_sb", tag="woT")
        wq_sb = persist.tile([P, c.CB, c.OD], BF16, name="wq_sb", tag="wq")

        nc.gpsimd.memset(v_sb[:, :, :, c.DH : c.DH + 1], 1.0)

        xT_r = xT.rearrange("(cb p) t -> p cb t", p=P)
        wq_r = wq.rearrange("(cb p) o -> p cb o", p=P)
        wk_r = wk.rearrange("(cb p) o -> p cb o", p=P)
        wv_r = wv.rearrange("(cb p) o -> p cb o", p=P)

        wkvp = ctx.enter_context(tc.tile_pool(name="wkvp", bufs=1))
        ep = ctx.enter_context(tc.tile_pool(name="ep", bufs=12))
        mp = ctx.enter_context(tc.tile_pool(name="mp", bufs=2))
        xp = ctx.enter_context(tc.tile_pool(name="xp", bufs=3))
        xk2 = ctx.enter_context(tc.tile_pool(name="xk2", bufs=2))
        xq = ctx.enter_context(tc.tile_pool(name="xq", bufs=1))
        ap = ctx.enter_context(tc.tile_pool(name="ap", bufs=2))
        atp = ctx.enter_context(tc.tile_pool(name="atp", bufs=2))
        op = ctx.enter_context(tc.tile_pool(name="op", bufs=2))
        rp = ctx.enter_context(tc.tile_pool(name="rp", bufs=4))

        wk_sb = wkvp.tile([P, c.CB, c.OD], BF16, name="wk_sb", tag="wk")
        wv_sb = wkvp.tile([P, c.CB, c.OD], BF16, name="wv_sb", tag="wv")
        ones_sb = wkvp.tile([P, 4 * c.ICSZ], BF16, name="ones_sb", tag="one")
        nc.gpsimd.memset(ones_sb, 1.0)
        nc.sync.dma_start(out=wk_sb, in_=wk_r)
        nc.sync.dma_start(out=wq_sb, in_=wq_r)
        nc.sync.dma_start(out=wv_sb, in_=wv_r)
        nc.sync.dma_start(
            out=woT_sb, in_=woT.rearrange("(ob p) n -> p ob n", p=P)
        )

        def kq_piece(dst8, w_sb, blk, x_t, tdst):
            """One (g2, kt) column-block x one t-chunk of the K/Q projection,
            landed as fp8 DoubleRow layout."""
            g2, kt = blk // 2, blk % 2
            ps = smp.tile([P, c.TCH], F32, name="ps_kq", tag="sm")
            for cb in range(c.CB):
                nc.tensor.matmul(
                    ps,
                    w_sb[:, cb, bass.ts(blk, P)],
                    x_t[:, cb, :],
                    start=(cb == 0),
                    stop=(cb == c.CB - 1),
                )
            nc.vector.tensor_copy(
                out=dst8[:, g2, kt, bass.ts(tdst, c.TCH)], in_=ps
            )

        def v_piece(x_t, tch):
            """V projection for one t-chunk (2 t-blocks of 128)."""
            for tbl in range(c.TCH // P):
                ps_v = smp.tile([P, c.OD], F32, name="ps_v", tag="sm")
                for cb in range(c.CB):
                    nc.tensor.matmul(
                        ps_v,
                        x_t[:, cb, bass.ts(tbl, P)],
                        wv_sb[:, cb, :],
                        start=(cb == 0),
                        stop=(cb == c.CB - 1),
                    )
                tb = tch * (c.TCH // P) + tbl
                nc.vector.tensor_copy(
                    out=v_sb[:, tb, :, 0 : c.DH],
                    in_=ps_v.rearrange("p (h d) -> p h d", h=c.NH),
                )

        def emit_scores(ic, h):
            """DoubleRow fp8 scores, then exp: j-groups [6, 6] on ScalarE
            (exact table exp) and the last 4 j-blocks on DVE via the custom
            poly-exp pair."""
            g2, h4 = h // 4, h % 4
            rows = slice(32 * h4, 32 * (h4 + 1))
            e = ep.tile([P, c.JB, c.ICSZ], BF16, name=f"e_{ic}_{h}", tag="e")
            jb0 = 0
            for gi, jj in enumerate(JGROUPS):
                ps = mmp.tile([P, 6, c.ICSZ], F32, name="ps_s", tag="mm")
                for j in range(jj):
                    nc.tensor.matmul(
                        ps[:, j, :],
                        k8[rows, g2, :, bass.ts(jb0 + j, P)],
                        q8[rows, g2, :, bass.ts(ic, c.ICSZ)],
                        start=True,
                        stop=True,
                        perf_mode=DR,
                        tile_position=(32 * h4, 0),
                    )
                use_dve = True
                if gi < 2 or not use_dve:
                    nc.scalar.activation(
                        out=e[:, jb0 : jb0 + jj, :],
                        in_=ps[:, 0:jj, :],
                        func=mybir.ActivationFunctionType.Exp,
                        scale=c.SCALE,
                    )
                else:
                    mid = mp.tile([P, jj * c.ICSZ], F16, name="mid", tag="mid")
                    nc.vector._custom_dve(
                        EXP2_POLY_ANT, out=mid, in0=ps[:, 0:jj, :],
                        in1=ones_sb, s0=EXP_C3, s1=EXP_C2, imm2=EXP_C1,
                    )
                    nc.vector._custom_dve(
                        EXP2_FIN_ANT, out=e[:, jb0 : jb0 + jj, :],
                        in0=mid, s0=EXP_CAP,
                    )
                jb0 += jj
            return e

        attn_tiles = {}
        attnT_tiles = {}

        def emit_av(ic, h, e):
            """attn[i, dh] = norm(e.T @ [v|1]) for head h."""
            if ic not in attn_tiles:
                attn_tiles[ic] = ap.tile(
                    [P, c.IB, c.NH, c.DH], BF16, name=f"attn_{ic}", tag="attn"
                )
            attn_sb = attn_tiles[ic]
            for ib in range(c.IB):
                ps_av = smp.tile([P, c.OD], F32, name="ps_av", tag="sm")
                for jb in range(c.JB):
                    nc.tensor.matmul(
                        ps_av[:, 0:VW],
                        e[:, jb, bass.ts(ib, P)],
                        v_sb[:, jb, h, :],
                        start=(jb == 0),
                        stop=(jb == c.JB - 1),
                    )
                rec = rp.tile([P, 1], F32, name="rec", tag="rec")
                nc.vector.reciprocal(rec, ps_av[:, c.DH : c.DH + 1])
                nc.vector.tensor_scalar_mul(
                    out=attn_sb[:, ib, h, :],
                    in0=ps_av[:, 0 : c.DH],
                    scalar1=rec,
                )

        def emit_transpose(ic):
            """DMA-transpose (XBAR) attn chunk ic into [od, i] layout."""
            attnT_tiles[ic] = atp.tile(
                [P, c.OB, c.ICSZ], BF16, name=f"attnT_{ic}", tag="attnT"
            )
            attnT_sb = attnT_tiles[ic]
            attn_sb = attn_tiles.pop(ic)
            for ib in range(c.IB):
                for hp in range(c.OB):
                    nc.sync.dma_start_transpose(
                        out=attnT_sb[:, hp, bass.ts(ib, P)],
                        in_=attn_sb[:, ib, 2 * hp : 2 * hp + 2, :],
                    )

        def emit_proj_piece(ic, tb):
            """Project one 128-query block of a finished chunk + store."""
            attnT_sb = attnT_tiles[ic]
            t0 = ic * c.ICSZ + tb * P
            for occ in range(2):
                ps_o = smp.tile([P, c.DIM // 2], F32, name="ps_o", tag="sm")
                for ob in range(c.OB):
                    nc.tensor.matmul(
                        ps_o,
                        attnT_sb[:, ob, bass.ts(tb, P)],
                        woT_sb[:, ob, bass.ts(occ, c.DIM // 2)],
                        start=(ob == 0),
                        stop=(ob == c.OB - 1),
                    )
                o_sb = op.tile([P, c.DIM // 2], F32, name="o_sb", tag="ost")
                nc.vector.tensor_copy(out=o_sb, in_=ps_o)
                nc.sync.dma_start(
                    out=out[t0 : t0 + P, bass.ts(occ, c.DIM // 2)], in_=o_sb
                )
            if tb == c.IB - 1:
                attnT_tiles.pop(ic)

        # ---------------- phase A: K(g2=0) + Q0 (all blocks) ----------------
        for tch in range(c.NTCH):
            x_t = xp.tile([P, c.CB, c.TCH], BF16, name=f"x_{tch}", tag="x")
            nc.sync.dma_start(out=x_t, in_=xT_r[:, :, bass.ts(tch, c.TCH)])
            kq_piece(k8, wk_sb, 0, x_t, tch)
            kq_piece(k8, wk_sb, 1, x_t, tch)
            if tch == 0:
                for blk in range(4):
                    kq_piece(q8, wq_sb, blk, x_t, 0)

        # prefetch x chunks for the K(g2=1) pass riding units 0-3
        xk_tiles = {}

        def xk_dma(tch):
            x_t = xk2.tile([P, c.CB, c.TCH], BF16, name=f"xk_{tch}", tag="xk")
            nc.sync.dma_start(out=x_t, in_=xT_r[:, :, bass.ts(tch, c.TCH)])
            xk_tiles[tch] = x_t

        xk_dma(0)
        xk_dma(1)

        # ---------------- main loop: 64 units of (ic, h) ----------------
        av_queue = []      # (ic, h, e) awaiting AV emission (gated on V)
        late_q = []        # PE work that trails: proj pieces
        v_done_unit = 10   # V pieces ride units 4..10
        x2_cur = None

        units = [(ic, h) for ic in range(c.NIC) for h in range(c.NH)]
        for u, (ic, h) in enumerate(units):
            e = emit_scores(ic, h)

            if u < 4:
                # K(g2=1): two t-chunks per unit
                for tch in (2 * u, 2 * u + 1):
                    kq_piece(k8, wk_sb, 2, xk_tiles[tch], tch)
                    kq_piece(k8, wk_sb, 3, xk_tiles[tch], tch)
                    if tch + 2 < c.NTCH:
                        xk_dma(tch + 2)
                    else:
                        # re-stream x for the V pass (same ring)
                        xk_dma(tch + 2 - c.NTCH)
            elif u <= v_done_unit:
                # V: 8 t-chunks over units 4..v_done_unit
                nv = v_done_unit - 3
                lo = (u - 4) * c.NTCH // nv
                hi = (u - 3) * c.NTCH // nv
                for tch in range(lo, hi):
                    v_piece(xk_tiles[tch], tch)
                    if tch + 2 < c.NTCH:
                        xk_dma(tch + 2)

            # JIT q for chunk ic+1: x2 DMA at h==0, blocks at odd h
            if ic + 1 < c.NIC:
                if h == 0:
                    x2_cur = xq.tile([P, c.CB, c.TCH], BF16, name="x2", tag="xq")
                    nc.sync.dma_start(
                        out=x2_cur, in_=xT_r[:, :, bass.ts(ic + 1, c.TCH)]
                    )
                if h % 2 == 1:
                    kq_piece(q8, wq_sb, h // 2, x2_cur, ic + 1)

            av_queue.append((ic, h, e))

            if u > v_done_unit:
                # drain up to 2 AVs + 1 trailing proj piece per unit
                for _ in range(2):
                    if av_queue:
                        aic, ah, ae = av_queue.pop(0)
                        emit_av(aic, ah, ae)
                        if ah == c.NH - 1:
                            emit_transpose(aic)
                            late_q.extend(
                                (aic, tb) for tb in range(c.IB)
                            )
                if late_q:
                    pic, tb = late_q.pop(0)
                    emit_proj_piece(pic, tb)

        # ---------------- drain ----------------
        while av_queue:
            aic, ah, ae = av_queue.pop(0)
            emit_av(aic, ah, ae)
            if ah == c.NH - 1:
                emit_transpose(aic)
                late_q.extend((aic, tb) for tb in range(c.IB))
        while late_q:
            pic, tb = late_q.pop(0)
            emit_proj_piece(pic, tb)


def build_nc(cfg: Cfg = Cfg(), reps: int = 1):
    nc = bacc.Bacc()
    xT = nc.declare_dram_parameter("xT", [cfg.DIM, cfg.T], BF16, isOutput=False)
    wq = nc.declare_dram_parameter("wq", [cfg.DIM, cfg.OD], BF16, isOutput=False)
    wk = nc.declare_dram_parameter("wk", [cfg.DIM, cfg.OD], BF16, isOutput=False)
    wv = nc.declare_dram_parameter("wv", [cfg.DIM, cfg.OD], BF16, isOutput=False)
    woT = nc.declare_dram_parameter("woT", [cfg.OD, cfg.DIM], BF16, isOutput=False)
    out = nc.declare_dram_parameter("out", [cfg.T, cfg.DIM], F32, isOutput=True)
    with tile.TileContext(nc) as tc:
        for _ in range(reps):
            _emit_kernel(tc, cfg, xT[:], wq[:], wk[:], wv[:], woT[:], out[:])
    nc.finalize()
    return nc


def prepare_core_inputs(x, w_qkv, w_out, b, g, cfg: Cfg, n_groups: int):
    """Host-side shard prep for core (batch b, head-group g)."""
    import ml_dtypes

    H = cfg.NH * n_groups
    heads = np.arange(cfg.NH * g, cfg.NH * (g + 1))
    bf16 = ml_dtypes.bfloat16

    # w_qkv row for (k, head h, dim d) is d*(3*H) + k*H + h
    def gather_perm(k_idx):
        # DoubleRow-permuted columns: c = g2*256 + kt*128 + h4*32 + dlo,
        # head h = heads[g2*4 + h4], d = kt*32 + dlo
        cols = np.empty(cfg.OD, dtype=np.int64)
        for g2 in range(2):
            for kt in range(2):
                for h4 in range(4):
                    h = heads[g2 * 4 + h4]
                    d = kt * 32 + np.arange(32)
                    c0 = g2 * 256 + kt * 128 + h4 * 32
                    cols[c0 : c0 + 32] = d * (3 * H) + k_idx * H + h
        return np.ascontiguousarray(w_qkv[cols, :].T).astype(bf16)

    def gather_std(k_idx):
        d = np.arange(cfg.DH)
        rows = (d[None, :] * (3 * H) + k_idx * H + heads[:, None]).reshape(-1)
        return np.ascontiguousarray(w_qkv[rows, :].T).astype(bf16)

    return {
        "xT": np.ascontiguousarray(x[b].T).astype(bf16),
        "wq": gather_perm(0),
        "wk": gather_perm(1),
        "wv": gather_std(2),
        "woT": np.ascontiguousarray(
            w_out[:, cfg.OD * g : cfg.OD * (g + 1)].T
        ).astype(bf16),
    }


_NC_CACHE = {}


def _get_nc(cfg: Cfg):
    if cfg not in _NC_CACHE:
        _NC_CACHE[cfg] = build_nc(cfg)
    return _NC_CACHE[cfg]


def run(x, w_qkv, w_out, b_out, trace=False):
    """Shard, execute on 8 cores, gather. Returns (out, BassKernelResults)."""
    cfg = Cfg()
    B, T, DIM = x.shape
    assert (T, DIM) == (cfg.T, cfg.DIM), (x.shape, cfg)
    n_groups = 2
    nc = _get_nc(cfg)
    in_maps = [
        prepare_core_inputs(x, w_qkv, w_out, b, g, cfg, n_groups)
        for b in range(B)
        for g in range(n_groups)
    ]
    res = run_bass_kernel_spmd(
        nc, in_maps, core_ids=list(range(len(in_maps))), trace=trace
    )
    out = np.empty((B, T, DIM), dtype=np.float32)
    for b in range(B):
        out[b] = res.results[2 * b]["out"] + res.results[2 * b + 1]["out"]
    out += b_out.astype(np.float32)
    return out, res


def _make_pjrt_fn(nc, in_maps):
    """Build a non-donating jitted 8-core runner for a prebuilt nc."""
    import jax
    import numpy as np_
    from jax.sharding import Mesh, PartitionSpec
    from jax.experimental.shard_map import shard_map

    from concourse import bass2jax

    bass2jax.install_neuronx_cc_hook()
    n_cores = len(in_maps)
    partition_name = nc.partition_id_tensor.name if nc.partition_id_tensor else None
    in_names, out_names, out_avals, zero_outs = [], [], [], []
    for alloc in nc.m.functions[0].allocations:
        if not isinstance(alloc, mybir.MemoryLocationSet):
            continue
        name = alloc.memorylocations[0].name
        if alloc.kind == "ExternalInput":
            if name != partition_name:
                in_names.append(name)
        elif alloc.kind == "ExternalOutput":
            shape = tuple(alloc.tensor_shape)
            dtype = mybir.dt.np(alloc.dtype)
            out_names.append(name)
            out_avals.append(jax.core.ShapedArray(shape, dtype))
            zero_outs.append(np_.zeros(shape, dtype))
    n_params = len(in_names)
    all_in_names = in_names + out_names
    if partition_name is not None:
        all_in_names = all_in_names + [partition_name]

    def _body(*args):
        operands = list(args)
        if partition_name is not None:
            operands.append(bass2jax.partition_id_tensor())
        return tuple(
            bass2jax._bass_exec_p.bind(
                *operands,
                out_avals=tuple(out_avals),
                in_names=tuple(all_in_names),
                out_names=tuple(out_names),
                lowering_input_output_aliases=(),
                sim_require_finite=True,
                sim_require_nnan=True,
                nc=nc,
            )
        )

    devices = jax.devices()[:n_cores]
    mesh = Mesh(np_.asarray(devices), ("core",))
    nin = n_params + len(out_names)
    f = jax.jit(
        shard_map(
            _body,
            mesh=mesh,
            in_specs=(PartitionSpec("core"),) * nin,
            out_specs=(PartitionSpec("core"),) * len(out_names),
            check_rep=False,
        ),
        keep_unused=True,
    )
    concat_in = [
        np_.concatenate([np_.asarray(in_maps[c][n]) for c in range(n_cores)], axis=0)
        for n in in_names
    ] + [np_.zeros((n_cores * z.shape[0], *z.shape[1:]), z.dtype) for z in zero_outs]
    dev_in = jax.device_put(concat_in)
    return f, dev_in


def _time_fn(f, dev_in, calls=4, rounds=6):
    import time

    import jax

    r = f(*dev_in)
    jax.block_until_ready(r)
    best = float("inf")
    for _ in range(rounds):
        t0 = time.perf_counter()
        rs = [f(*dev_in) for _ in range(calls)]
        jax.block_until_ready(rs)
        best = min(best, (time.perf_counter() - t0) / calls)
    return best


def time_hw(x, w_qkv, w_out, b_out, reps=(4, 36)):
    """Marginal-cost HW timing: per-call time of an R2-repeat NEFF minus an
    R1-repeat NEFF, over (R2-R1), cancels the axon dispatch overhead."""
    cfg = Cfg()
    B = x.shape[0]
    in_maps = [
        prepare_core_inputs(x, w_qkv, w_out, b, g, cfg, 2)
        for b in range(B)
        for g in range(2)
    ]
    r1, r2 = reps
    ncA = build_nc(cfg, reps=r1)
    fA, devA = _make_pjrt_fn(ncA, in_maps)
    tA = _time_fn(fA, devA)
    ncB = build_nc(cfg, reps=r2)
    fB, devB = _make_pjrt_fn(ncB, in_maps)
    tB = _time_fn(fB, devB)
    per_exec = (tB - tA) / (r2 - r1)
    return tA, per_exec


def kernel(x, w_qkv, w_out, b_out):
    x = np.asarray(x, dtype=np.float32)
    w_qkv = np.asarray(w_qkv, dtype=np.float32)
    w_out = np.asarray(w_out, dtype=np.float32)
    b_out = np.asarray(b_out, dtype=np.float32)
    try:
        out, _ = run(x, w_qkv, w_out, b_out, trace=False)
    except Exception:
        # one retry for transient device errors
        out, _ = run(x, w_qkv, w_out, b_out, trace=False)
    return out
